# revision 14
# baseline (speedup 1.0000x reference)
"""ContextQueryAttention Trainium2 kernel (fp8 DoubleRow E-side).

Reference computation (per batch b):
    S = (c@wc)[:,None] + (q@wq)[None,:] + (c*wm) @ q.T        # (Lc, Lq)
    S1 = softmax(S, axis=0)  (over context dim i)
    S2 = softmax(S, axis=1)  (over question dim j)
    A  = S1 @ q
    Bm = (S1 @ S2.T) @ c
    out = [c, A, c*A, c*Bm] @ wr + br

Algebraic restructuring (as the bf16 baseline):
  * Bm = S1 @ (S2.T @ c); q~ = wm*q + wc so E1T = exp(q~ @ c.T) gives the
    i-softmax numerators directly (v[j] term cancels); softmax normalizers
    folded into downstream operands.

This version:
  * Scores in bf16 (numerically equivalent to f32r at the output metric).
  * E1T, G=S2, q/s1, c, Y held in float8e4 with static power-of-2 scales;
    the S2^T@c, A, Bm matmuls and the s2 reduction run on the PE in fp8,
    the big contractions in DoubleRow mode (K=256/instr, 2x bf16 rate).
  * Final (Lc,4D)@(4D,D) projection in bf16 (fp8 would breach the error
    budget), bias added via a broadcast tile on the DVE instead of rank-1
    matmuls.
  * softmax reciprocals on column-form [128,*] tiles (the row-form
    reciprocal was a 6.4us single-lane DVE op); ev row->column via PE
    transposes instead of a DRAM roundtrip; the s2 roundtrip is hidden
    behind the A^T matmuls.

Sharding: pure data parallel over batch: 16 batches -> 8 cores x 2.
"""

import numpy as np

import concourse.bass as bass
import concourse.tile as tile
from concourse import bacc, mybir
from concourse import bass2jax
from concourse.masks import make_identity

N_CORES = 8
B, Lc, Lq, D = 16, 2048, 512, 512
BPC = B // N_CORES  # batches per core

F32 = mybir.dt.float32
F32R = mybir.dt.float32r
BF16 = mybir.dt.bfloat16
FP8 = mybir.dt.float8e4
DRM = mybir.MatmulPerfMode.DoubleRow

AF = mybir.ActivationFunctionType
ALU = mybir.AluOpType
AX = mybir.AxisListType

NT = Lc // 128   # 16 context row-blocks
NG = Lq // 128   # 4 question row-blocks
NK = D // 128    # 4 feature blocks
NC = Lc // 512   # 4 i-chunks of 512

# static power-of-2 scales keeping fp8 operands in e4m3's sweet spot
SEV = 2.0      # ev
SQ = 4096.0    # q/s1
SG = 512.0     # G = S2 columns
SY = 8192.0    # Y


def build_program(repeat=1):
    nc = bacc.Bacc(None, target_bir_lowering=False)

    c2 = nc.declare_dram_parameter("c2", [BPC, Lc, D], F32, isOutput=False)
    q2 = nc.declare_dram_parameter("q2", [BPC, Lq, D], F32, isOutput=False)
    w0 = nc.declare_dram_parameter("w0", [3 * D], F32, isOutput=False)
    wr = nc.declare_dram_parameter("wr", [4 * D, D], F32, isOutput=False)
    br = nc.declare_dram_parameter("br", [D], F32, isOutput=False)
    out2 = nc.declare_dram_parameter("out2", [BPC, Lc, D], F32, isOutput=True)

    def load(out, in_):
        # gpsimd (SWDGE) DMAs cast f32 -> bf16/fp8 on the fly
        nc.gpsimd.dma_start(out=out, in_=in_)

    with tile.TileContext(nc) as tc:
        with (
            tc.tile_pool(name="sb", bufs=1) as sb,
            tc.tile_pool(name="ps", bufs=2, space="PSUM") as ps,
            tc.tile_pool(name="pt", bufs=2, space="PSUM") as pt,
            tc.tile_pool(name="p8", bufs=2, space="PSUM") as p8,
        ):
            # ---- constants ----
            ident_f = sb.tile([128, 128], F32, tag="identf")
            make_identity(nc, ident_f)
            identb = sb.tile([128, 128], BF16, tag="identb")
            nc.vector.tensor_copy(identb, ident_f)
            ident8 = sb.tile([128, 128], FP8, tag="ident8")
            nc.vector.tensor_copy(ident8, ident_f)

            def issue_loads(b):
                # q (casting SWDGE) first, c raw f32 via fast HWDGE,
                # c fp8 via casting SWDGE
                qSt = sb.tile([128, NG, D], F32, tag="qSt", bufs=2)
                nc.scalar.dma_start(out=qSt,
                                    in_=q2[b].rearrange("(g p) d -> p g d", p=128))
                qB = sb.tile([128, NG, D], BF16, tag="qB", bufs=2)
                nc.vector.tensor_copy(qB, qSt)
                cSt = sb.tile([128, NT, D], F32, tag="cSt")
                c_r = c2[b].rearrange("(t p) d -> p t d", p=128)
                for tq in range(4):
                    eng = nc.sync if tq < 2 else nc.scalar
                    eng.dma_start(out=cSt[:, tq * 4:(tq + 1) * 4, :],
                                  in_=c_r[:, tq * 4:(tq + 1) * 4, :])
                c8 = sb.tile([128, NT, D], FP8, tag="c8")
                for tq in range(4):
                    load(c8[:, tq * 4:(tq + 1) * 4, :],
                         c_r[:, tq * 4:(tq + 1) * 4, :])
                return qB, cSt, c8

            wc_sb = sb.tile([128, NK], F32, tag="wc")
            wm_sb = sb.tile([128, NK], F32, tag="wm")
            wq_sb = sb.tile([128, NK], BF16, tag="wq")
            load(wq_sb, w0[D:2 * D].rearrange("(k p) -> p k", p=128))
            nc.sync.dma_start(out=wc_sb, in_=w0[0:D].rearrange("(k p) -> p k", p=128))
            nc.sync.dma_start(out=wm_sb, in_=w0[2 * D:3 * D].rearrange("(k p) -> p k", p=128))

            # bias broadcast tile: br_bc[p, e] = br[e] for all p
            br_row = sb.tile([1, D], F32, tag="brrow")
            nc.sync.dma_start(out=br_row, in_=br.rearrange("(a e) -> a e", a=1))

            tiles0 = issue_loads(0)

            W_sb = sb.tile([128, 4 * NK, D], BF16, tag="W")
            wr_r = wr.rearrange("(t p) e -> p t e", p=128)
            for tq in range(4):
                load(W_sb[:, tq * NK:(tq + 1) * NK, :],
                     wr_r[:, tq * NK:(tq + 1) * NK, :])
            ones_f = sb.tile([1, 128], F32, tag="onesf")
            nc.vector.memset(ones_f, 1.0)
            pbr = pt.tile([128, 512], F32, tag="tr")
            nc.tensor.matmul(pbr[:, 0:D], ones_f, br_row, start=True, stop=True)
            br_bc = sb.tile([128, D], F32, tag="brbc")
            nc.any.tensor_copy(br_bc, pbr[:, 0:D])

            def one_batch(b, tiles):
                qB, cSt, c8 = tiles

                # ---- q transposes: qTb[d%128, kd, j] ----
                qTb = sb.tile([128, NK, Lq], BF16, tag="qTb")
                for kd in range(NK):
                    ptile = pt.tile([128, 1024], BF16, tag="tr")
                    for g in range(NG):
                        nc.tensor.transpose(
                            ptile[:, g * 128:(g + 1) * 128],
                            qB[:, g, kd * 128:(kd + 1) * 128], identb)
                    nc.any.tensor_copy(qTb[:, kd, :], ptile[:, 0:Lq])

                # ---- v = q @ wq (row form) -> ev column form via PE ----
                pv = ps.tile([128, 512], F32, tag="mm")
                for kd in range(NK):
                    nc.tensor.matmul(pv[0:1, :], wq_sb[:, kd:kd + 1], qTb[:, kd, :],
                                     start=(kd == 0), stop=(kd == NK - 1))
                ev_row = sb.tile([1, Lq], F32, tag="evrow")
                nc.scalar.activation(out=ev_row, in_=pv[0:1, :], func=AF.Exp)
                ptev = pt.tile([128, 512], F32, tag="tr")
                for g in range(NG):
                    nc.tensor.transpose(ptev[:, g:g + 1],
                                        ev_row[0:1, g * 128:(g + 1) * 128],
                                        ident_f[0:1, 0:1])
                ev_colf = sb.tile([128, NG], F32, tag="evcolf")
                nc.any.tensor_copy(ev_colf, ptev[:, 0:NG])
                ev8 = sb.tile([128, NG, 1], FP8, tag="ev8")
                nc.vector.tensor_scalar_mul(ev8[:, :, 0], ptev[:, 0:NG], SEV)

                # ---- c transposes (f32r): cTb[d%128, kd, i] ----
                cTb = sb.tile([128, NK, Lc], BF16, tag="cTb")
                for kd in range(NK):
                    for ic in range(4):
                        ptile = pt.tile([128, 512], F32, tag="tr")
                        for t4 in range(4):
                            t = ic * 4 + t4
                            nc.tensor.transpose(
                                ptile[:, t4 * 128:(t4 + 1) * 128],
                                cSt[:, t, kd * 128:(kd + 1) * 128], ident_f)
                        nc.any.tensor_copy(cTb[:, kd, ic * 512:(ic + 1) * 512], ptile)

                # ---- q~T = wm * qT + wc (in place) ----
                for kd in range(NK):
                    nc.vector.tensor_scalar(
                        out=qTb[:, kd, :], in0=qTb[:, kd, :],
                        scalar1=wm_sb[:, kd:kd + 1], scalar2=wc_sb[:, kd:kd + 1],
                        op0=ALU.mult, op1=ALU.add)

                # ---- scores + exp -> E (fp8, DR-pair layout) + s1 ----
                # E[j%128, gp, ch, gi, col]: chunk ch of 512 i's, g = 2*gp+gi
                E = sb.tile([128, 2, NC, 2, 512], FP8, tag="E")
                s1p = sb.tile([128, NG, 2], F32, tag="s1p")
                s1s = sb.tile([128, NG], F32, tag="s1s")
                invs1 = sb.tile([128, NG], F32, tag="invs1")
                invs1q = sb.tile([128, NG], F32, tag="invs1q")
                for g in range(NG):
                    gp, gi = g // 2, g % 2
                    for ic2 in range(2):
                        pm = ps.tile([128, 1024], F32, tag="mm")
                        for half in range(2):
                            ic = ic2 * 2 + half
                            for kd in range(NK):
                                nc.tensor.matmul(
                                    pm[:, half * 512:(half + 1) * 512],
                                    qTb[:, kd, g * 128:(g + 1) * 128],
                                    cTb[:, kd, ic * 512:(ic + 1) * 512],
                                    start=(kd == 0), stop=(kd == NK - 1))
                        nc.scalar.activation(
                            out=E[:, gp, 2 * ic2:2 * ic2 + 2, gi, :], in_=pm,
                            func=AF.Exp, accum_out=s1p[:, g, ic2:ic2 + 1])
                    nc.vector.reduce_sum(out=s1s[:, g:g + 1], in_=s1p[:, g, :], axis=AX.X)
                    nc.vector.reciprocal(out=invs1[:, g:g + 1], in_=s1s[:, g:g + 1])
                nc.vector.tensor_scalar_mul(invs1q, invs1, SQ)

                # ---- s2 row (rank-1 fp8 matmuls) -> column form via PE ----
                s2row = sb.tile([1, Lc], F32, tag="s2row")
                for ch in range(NC):
                    s2p = ps.tile([128, 512], F32, tag="mm")
                    for g in range(NG):
                        gp, gi = g // 2, g % 2
                        nc.tensor.matmul(
                            s2p[0:1, :], ev8[:, g:g + 1, 0:1], E[:, gp, ch, gi, :],
                            start=(g == 0), stop=(g == NG - 1))
                    nc.any.tensor_copy(s2row[0:1, ch * 512:(ch + 1) * 512],
                                       s2p[0:1, :])
                pts2 = pt.tile([128, 512], F32, tag="tr")
                for t in range(NT):
                    nc.tensor.transpose(pts2[:, t:t + 1],
                                        s2row[0:1, t * 128:(t + 1) * 128],
                                        ident_f[0:1, 0:1])

                # ---- qN8[j%128, kd, g, :] = q * invs1 * SQ (fp8) ----
                qN8 = sb.tile([128, NK, NG, 128], FP8, tag="qN8")
                for g in range(NG):
                    nc.vector.tensor_scalar_mul(qN8[:, :, g, :], qB[:, g, :],
                                                invs1q[:, g:g + 1])

                # ---- AT = (q/s1).T @ E (DR); cAT = cTb * AT ----
                AT = sb.tile([128, NK, Lc], BF16, tag="AT")
                cAT = sb.tile([128, NK, Lc], BF16, tag="cAT")
                for kd in range(NK):
                    for ch in range(NC):
                        pm = ps.tile([128, 512], F32, tag="mm")
                        for gp in range(2):
                            nc.tensor.matmul(
                                pm, qN8[:, kd, 2 * gp:2 * gp + 2, :],
                                E[:, gp, ch, :, :],
                                start=(gp == 0), stop=(gp == 1), perf_mode=DRM)
                        sl = slice(ch * 512, (ch + 1) * 512)
                        nc.any.tensor_scalar_mul(AT[:, kd, sl], pm, 1.0 / SQ)
                        nc.gpsimd.tensor_mul(cAT[:, kd, sl], cTb[:, kd, sl], AT[:, kd, sl])

                # ---- G transposes + scale: G[i%128, jg, t, :] = E^T/s2*SG ----
                invs2c = sb.tile([128, NT], F32, tag="invs2c")
                nc.vector.reciprocal(out=invs2c, in_=pts2[:, 0:NT])
                nc.vector.tensor_scalar_mul(invs2c, invs2c, SG * SEV)
                G = sb.tile([128, NG, NT, 128], FP8, tag="G")
                for tp in range(NT // 2):
                    p8t = p8.tile([128, NG, 2, 128, 2], FP8, tag="tr8")
                    for tl in range(2):
                        t = 2 * tp + tl
                        ch, off = t // 4, (t % 4) * 128
                        for g in range(NG):
                            gp, gi = g // 2, g % 2
                            nc.tensor.transpose(
                                p8t[:, g, tl, :, 0],
                                E[:, gp, ch, gi, off:off + 128], ident8)
                    for tl in range(2):
                        t = 2 * tp + tl
                        nc.any.tensor_scalar_mul(G[:, :, t, :],
                                                 p8t[:, :, tl, :, 0],
                                                 invs2c[:, t:t + 1])

                # ---- Y8 = (G.T @ c8) * ev/s1 * SY (DR) ----
                ysc = sb.tile([128, NG], F32, tag="ysc")
                nc.vector.tensor_mul(ysc, ev_colf, invs1)
                nc.vector.tensor_scalar_mul(ysc, ysc, SY / SG)
                Y8 = sb.tile([128, NK, NG, 128], FP8, tag="Y8")
                for g in range(NG):
                    pm = ps.tile([128, 512], F32, tag="mm")
                    for tp in range(NT // 2):
                        nc.tensor.matmul(
                            pm, G[:, g, 2 * tp:2 * tp + 2, :],
                            c8[:, 2 * tp:2 * tp + 2, :],
                            start=(tp == 0), stop=(tp == NT // 2 - 1),
                            perf_mode=DRM)
                    nc.vector.tensor_scalar_mul(Y8[:, :, g, :], pm,
                                                ysc[:, g:g + 1])

                # ---- BmT = Y.T @ E (DR); cBmT = cTb * BmT ----
                BmT = sb.tile([128, NK, Lc], BF16, tag="BmT")
                for kd in range(NK):
                    for ch in range(NC):
                        pm = ps.tile([128, 512], F32, tag="mm")
                        for gp in range(2):
                            nc.tensor.matmul(
                                pm, Y8[:, kd, 2 * gp:2 * gp + 2, :],
                                E[:, gp, ch, :, :],
                                start=(gp == 0), stop=(gp == 1), perf_mode=DRM)
                        sl = slice(ch * 512, (ch + 1) * 512)
                        nc.any.tensor_scalar_mul(BmT[:, kd, sl], pm, 1.0 / SY)
                        nc.gpsimd.tensor_mul(BmT[:, kd, sl], BmT[:, kd, sl], cTb[:, kd, sl])

                # ---- out = c@W1 + A@W2 + cA@W3 + cB@W4 + br ----
                for t2 in range(NT // 2):
                    pm = ps.tile([128, 1024], F32, tag="mm")
                    for half in range(2):
                        t = t2 * 2 + half
                        first = True
                        for si, src in enumerate((cTb, AT, cAT, BmT)):
                            for kd in range(NK):
                                nc.tensor.matmul(
                                    pm[:, half * 512:(half + 1) * 512],
                                    src[:, kd, t * 128:(t + 1) * 128],
                                    W_sb[:, si * NK + kd, :],
                                    start=first, stop=(si == 3 and kd == NK - 1))
                                first = False
                    ot = sb.tile([128, 2, 512], F32, tag="outst", bufs=3)
                    for half in range(2):
                        nc.any.tensor_add(ot[:, half, :],
                                          pm[:, half * 512:(half + 1) * 512], br_bc)
                    nc.sync.dma_start(
                        out=out2[b].rearrange("(u p) e -> p u e", p=128)[:, t2 * 2:t2 * 2 + 2, :],
                        in_=ot)

            if repeat > 1:
                hints = (mybir.EngineType.PE, mybir.EngineType.DVE,
                         mybir.EngineType.Activation, mybir.EngineType.SP,
                         mybir.EngineType.Pool)
                with tc.For_i(0, repeat, 1, hint_engines=hints):
                    for b in range(BPC):
                        one_batch(b, tiles0 if b == 0 else issue_loads(b))
            else:
                for b in range(BPC):
                    one_batch(b, tiles0 if b == 0 else issue_loads(b))

    nc.compile()
    return nc


class Runner:
    """Persistent SPMD runner: jit once, execute many times."""

    def __init__(self, nc):
        import jax
        from jax.experimental.shard_map import shard_map
        from jax.sharding import Mesh, PartitionSpec

        bass2jax.install_neuronx_cc_hook()
        self.nc = nc
        self.jax = jax

        partition_name = (
            nc.partition_id_tensor.name if nc.partition_id_tensor else None
        )
        in_names, out_names, out_avals, zero_shapes = [], [], [], []
        for alloc in nc.m.functions[0].allocations:
            if not isinstance(alloc, mybir.MemoryLocationSet):
                continue
            name = alloc.memorylocations[0].name
            if alloc.kind == "ExternalInput":
                if name != partition_name:
                    in_names.append(name)
            elif alloc.kind == "ExternalOutput":
                shape = tuple(alloc.tensor_shape)
                dtype = mybir.dt.np(alloc.dtype)
                out_names.append(name)
                out_avals.append(jax.core.ShapedArray(shape, dtype))
                zero_shapes.append((shape, dtype))
        self.in_names = list(in_names)
        self.out_names = out_names
        self.out_avals = out_avals
        self.zero_shapes = zero_shapes
        n_params = len(in_names)
        n_outs = len(out_names)

        all_in_names = list(in_names) + list(out_names)
        if partition_name is not None:
            all_in_names.append(partition_name)

        def _body(*args):
            operands = list(args)
            if partition_name is not None:
                operands.append(bass2jax.partition_id_tensor())
            outs = bass2jax._bass_exec_p.bind(
                *operands,
                out_avals=tuple(out_avals),
                in_names=tuple(all_in_names),
                out_names=tuple(out_names),
                lowering_input_output_aliases=(),
                sim_require_finite=True,
                sim_require_nnan=True,
                nc=nc,
            )
            return tuple(outs)

        devices = jax.devices()[:N_CORES]
        mesh = Mesh(np.asarray(devices), ("core",))
        in_specs = (PartitionSpec("core"),) * (n_params + n_outs)
        out_specs = (PartitionSpec("core"),) * n_outs
        self.fn = jax.jit(
            shard_map(_body, mesh=mesh, in_specs=in_specs,
                      out_specs=out_specs, check_rep=False),
            keep_unused=True,
        )

    def concat_inputs(self, in_maps):
        return [
            np.concatenate([np.asarray(m[name]) for m in in_maps], axis=0)
            for name in self.in_names
        ]

    def zeros(self):
        return [
            np.zeros((N_CORES * s[0], *s[1:]), d) for (s, d) in self.zero_shapes
        ]

    def run_device(self, concat_in, zeros):
        out = self.fn(*concat_in, *zeros)
        self.jax.block_until_ready(out)
        return out

    def run(self, in_maps):
        outs = self.run_device(self.concat_inputs(in_maps), self.zeros())
        return [
            {
                name: np.asarray(outs[i]).reshape(
                    N_CORES, *self.out_avals[i].shape)[c]
                for i, name in enumerate(self.out_names)
            }
            for c in range(N_CORES)
        ]


_CACHED = {}


def _get_runner(**kw):
    key = tuple(sorted(kw.items()))
    if key not in _CACHED:
        _CACHED[key] = Runner(build_program(**kw))
    return _CACHED[key]


def make_in_maps(context, question, w0, wr, br):
    return [
        {
            "c2": context[c * BPC:(c + 1) * BPC],
            "q2": question[c * BPC:(c + 1) * BPC],
            "w0": w0,
            "wr": wr,
            "br": br,
        }
        for c in range(N_CORES)
    ]


def kernel(context, question, w0, wr, br):
    context = np.ascontiguousarray(np.asarray(context, dtype=np.float32))
    question = np.ascontiguousarray(np.asarray(question, dtype=np.float32))
    w0 = np.ascontiguousarray(np.asarray(w0, dtype=np.float32))
    wr = np.ascontiguousarray(np.asarray(wr, dtype=np.float32))
    br = np.ascontiguousarray(np.asarray(br, dtype=np.float32))

    runner = _get_runner()
    res = runner.run(make_in_maps(context, question, w0, wr, br))
    return np.concatenate([res[c]["out2"] for c in range(N_CORES)], axis=0)


# revision 15
# speedup vs baseline: 1.0177x; 1.0177x over previous
"""ContextQueryAttention Trainium2 kernel (fp8 DoubleRow E-side).

Reference computation (per batch b):
    S = (c@wc)[:,None] + (q@wq)[None,:] + (c*wm) @ q.T        # (Lc, Lq)
    S1 = softmax(S, axis=0)  (over context dim i)
    S2 = softmax(S, axis=1)  (over question dim j)
    A  = S1 @ q
    Bm = (S1 @ S2.T) @ c
    out = [c, A, c*A, c*Bm] @ wr + br

Algebraic restructuring (as the bf16 baseline):
  * Bm = S1 @ (S2.T @ c); q~ = wm*q + wc so E1T = exp(q~ @ c.T) gives the
    i-softmax numerators directly (v[j] term cancels); softmax normalizers
    folded into downstream operands.

This version:
  * Scores in bf16 (numerically equivalent to f32r at the output metric).
  * E1T, G=S2, q/s1, c, Y held in float8e4 with static power-of-2 scales;
    the S2^T@c, A, Bm matmuls and the s2 reduction run on the PE in fp8,
    the big contractions in DoubleRow mode (K=256/instr, 2x bf16 rate).
  * Final (Lc,4D)@(4D,D) projection in bf16 (fp8 would breach the error
    budget), bias added via a broadcast tile on the DVE instead of rank-1
    matmuls.
  * softmax reciprocals on column-form [128,*] tiles (the row-form
    reciprocal was a 6.4us single-lane DVE op); ev row->column via PE
    transposes instead of a DRAM roundtrip; the s2 roundtrip is hidden
    behind the A^T matmuls.

Sharding: pure data parallel over batch: 16 batches -> 8 cores x 2.
"""

import numpy as np

import concourse.bass as bass
import concourse.tile as tile
from concourse import bacc, mybir
from concourse import bass2jax
from concourse.masks import make_identity

N_CORES = 8
B, Lc, Lq, D = 16, 2048, 512, 512
BPC = B // N_CORES  # batches per core

F32 = mybir.dt.float32
F32R = mybir.dt.float32r
BF16 = mybir.dt.bfloat16
FP8 = mybir.dt.float8e4
DRM = mybir.MatmulPerfMode.DoubleRow

AF = mybir.ActivationFunctionType
ALU = mybir.AluOpType
AX = mybir.AxisListType

NT = Lc // 128   # 16 context row-blocks
NG = Lq // 128   # 4 question row-blocks
NK = D // 128    # 4 feature blocks
NC = Lc // 512   # 4 i-chunks of 512

# static power-of-2 scales keeping fp8 operands in e4m3's sweet spot
SEV = 2.0      # ev
SQ = 4096.0    # q/s1
SG = 512.0     # G = S2 columns
SY = 8192.0    # Y


def build_program(repeat=1):
    nc = bacc.Bacc(None, target_bir_lowering=False)

    c2 = nc.declare_dram_parameter("c2", [BPC, Lc, D], F32, isOutput=False)
    q2 = nc.declare_dram_parameter("q2", [BPC, Lq, D], F32, isOutput=False)
    w0 = nc.declare_dram_parameter("w0", [3 * D], F32, isOutput=False)
    wr = nc.declare_dram_parameter("wr", [4 * D, D], F32, isOutput=False)
    br = nc.declare_dram_parameter("br", [D], F32, isOutput=False)
    out2 = nc.declare_dram_parameter("out2", [BPC, Lc, D], F32, isOutput=True)

    def load(out, in_):
        # gpsimd (SWDGE) DMAs cast f32 -> bf16/fp8 on the fly
        nc.gpsimd.dma_start(out=out, in_=in_)

    with tile.TileContext(nc) as tc:
        with (
            tc.tile_pool(name="sb", bufs=1) as sb,
            tc.tile_pool(name="ps", bufs=2, space="PSUM") as ps,
            tc.tile_pool(name="pt", bufs=2, space="PSUM") as pt,
            tc.tile_pool(name="p8", bufs=2, space="PSUM") as p8,
        ):
            # ---- constants ----
            ident_f = sb.tile([128, 128], F32, tag="identf")
            make_identity(nc, ident_f)
            identb = sb.tile([128, 128], BF16, tag="identb")
            nc.vector.tensor_copy(identb, ident_f)
            ident8 = sb.tile([128, 128], FP8, tag="ident8")
            nc.vector.tensor_copy(ident8, ident_f)

            def issue_loads(b):
                # q (casting SWDGE) first, c raw f32 via fast HWDGE,
                # c fp8 via casting SWDGE
                qSt = sb.tile([128, NG, D], F32, tag="qSt", bufs=2)
                nc.sync.dma_start(out=qSt,
                                  in_=q2[b].rearrange("(g p) d -> p g d", p=128))
                qB = sb.tile([128, NG, D], BF16, tag="qB", bufs=2)
                nc.vector.tensor_copy(qB, qSt)
                cSt = sb.tile([128, NT, D], F32, tag="cSt")
                c_r = c2[b].rearrange("(t p) d -> p t d", p=128)
                for tq in range(4):
                    eng = nc.sync if tq < 2 else nc.scalar
                    eng.dma_start(out=cSt[:, tq * 4:(tq + 1) * 4, :],
                                  in_=c_r[:, tq * 4:(tq + 1) * 4, :])
                c8 = sb.tile([128, NT, D], FP8, tag="c8")
                for tq in range(4):
                    load(c8[:, tq * 4:(tq + 1) * 4, :],
                         c_r[:, tq * 4:(tq + 1) * 4, :])
                return qB, cSt, c8

            wc_sb = sb.tile([128, NK], F32, tag="wc")
            wm_sb = sb.tile([128, NK], F32, tag="wm")
            wq_sb = sb.tile([128, NK], BF16, tag="wq")
            load(wq_sb, w0[D:2 * D].rearrange("(k p) -> p k", p=128))
            nc.sync.dma_start(out=wc_sb, in_=w0[0:D].rearrange("(k p) -> p k", p=128))
            nc.sync.dma_start(out=wm_sb, in_=w0[2 * D:3 * D].rearrange("(k p) -> p k", p=128))

            # bias broadcast tile: br_bc[p, e] = br[e] for all p
            br_row = sb.tile([1, D], F32, tag="brrow")
            nc.sync.dma_start(out=br_row, in_=br.rearrange("(a e) -> a e", a=1))

            tiles0 = issue_loads(0)

            W_sb = sb.tile([128, 4 * NK, D], BF16, tag="W")
            wr_r = wr.rearrange("(t p) e -> p t e", p=128)
            for tq in range(4):
                load(W_sb[:, tq * NK:(tq + 1) * NK, :],
                     wr_r[:, tq * NK:(tq + 1) * NK, :])
            ones_f = sb.tile([1, 128], F32, tag="onesf")
            nc.vector.memset(ones_f, 1.0)
            pbr = pt.tile([128, 512], F32, tag="tr")
            nc.tensor.matmul(pbr[:, 0:D], ones_f, br_row, start=True, stop=True)
            br_bc = sb.tile([128, D], F32, tag="brbc")
            nc.any.tensor_copy(br_bc, pbr[:, 0:D])

            def one_batch(b, tiles):
                qB, cSt, c8 = tiles

                # ---- q transposes: qTb[d%128, kd, j] ----
                qTb = sb.tile([128, NK, Lq], BF16, tag="qTb")
                for kd in range(NK):
                    ptile = pt.tile([128, 1024], BF16, tag="tr")
                    for g in range(NG):
                        nc.tensor.transpose(
                            ptile[:, g * 128:(g + 1) * 128],
                            qB[:, g, kd * 128:(kd + 1) * 128], identb)
                    nc.any.tensor_copy(qTb[:, kd, :], ptile[:, 0:Lq])

                # ---- v = q @ wq (row form) -> ev column form via PE ----
                pv = ps.tile([128, 512], F32, tag="mm")
                for kd in range(NK):
                    nc.tensor.matmul(pv[0:1, :], wq_sb[:, kd:kd + 1], qTb[:, kd, :],
                                     start=(kd == 0), stop=(kd == NK - 1))
                ev_row = sb.tile([1, Lq], F32, tag="evrow")
                nc.scalar.activation(out=ev_row, in_=pv[0:1, :], func=AF.Exp)

                # ---- c transposes (f32r): cTb[d%128, kd, i] ----
                cTb = sb.tile([128, NK, Lc], BF16, tag="cTb")
                for kd in range(NK):
                    for ic in range(4):
                        ptile = pt.tile([128, 512], F32, tag="tr")
                        for t4 in range(4):
                            t = ic * 4 + t4
                            nc.tensor.transpose(
                                ptile[:, t4 * 128:(t4 + 1) * 128],
                                cSt[:, t, kd * 128:(kd + 1) * 128], ident_f)
                        nc.any.tensor_copy(cTb[:, kd, ic * 512:(ic + 1) * 512], ptile)

                # ---- q~T = wm * qT + wc (in place) ----
                for kd in range(NK):
                    nc.vector.tensor_scalar(
                        out=qTb[:, kd, :], in0=qTb[:, kd, :],
                        scalar1=wm_sb[:, kd:kd + 1], scalar2=wc_sb[:, kd:kd + 1],
                        op0=ALU.mult, op1=ALU.add)

                # ---- scores + exp -> E (fp8, DR-pair layout) + s1 ----
                # E[j%128, gp, ch, gi, col]: chunk ch of 512 i's, g = 2*gp+gi
                E = sb.tile([128, 2, NC, 2, 512], FP8, tag="E")
                s1p = sb.tile([128, NG, 2], F32, tag="s1p")
                s1s = sb.tile([128, NG], F32, tag="s1s")
                invs1 = sb.tile([128, NG], F32, tag="invs1")
                invs1q = sb.tile([128, NG], F32, tag="invs1q")
                for g in range(NG):
                    gp, gi = g // 2, g % 2
                    for ic2 in range(2):
                        pm = ps.tile([128, 1024], F32, tag="mm")
                        for half in range(2):
                            ic = ic2 * 2 + half
                            for kd in range(NK):
                                nc.tensor.matmul(
                                    pm[:, half * 512:(half + 1) * 512],
                                    qTb[:, kd, g * 128:(g + 1) * 128],
                                    cTb[:, kd, ic * 512:(ic + 1) * 512],
                                    start=(kd == 0), stop=(kd == NK - 1))
                        nc.scalar.activation(
                            out=E[:, gp, 2 * ic2:2 * ic2 + 2, gi, :], in_=pm,
                            func=AF.Exp, accum_out=s1p[:, g, ic2:ic2 + 1])
                    nc.vector.reduce_sum(out=s1s[:, g:g + 1], in_=s1p[:, g, :], axis=AX.X)
                    nc.vector.reciprocal(out=invs1[:, g:g + 1], in_=s1s[:, g:g + 1])
                nc.vector.tensor_scalar_mul(invs1q, invs1, SQ)

                ptev = pt.tile([128, 512], F32, tag="tr")
                for g in range(NG):
                    nc.tensor.transpose(ptev[:, g:g + 1],
                                        ev_row[0:1, g * 128:(g + 1) * 128],
                                        ident_f[0:1, 0:1])
                ev_colf = sb.tile([128, NG], F32, tag="evcolf")
                nc.any.tensor_copy(ev_colf, ptev[:, 0:NG])
                ev8 = sb.tile([128, NG, 1], FP8, tag="ev8")
                nc.vector.tensor_scalar_mul(ev8[:, :, 0], ptev[:, 0:NG], SEV)

                # ---- s2 row (rank-1 fp8 matmuls) -> column form via PE ----
                s2row = sb.tile([1, Lc], F32, tag="s2row")
                for ch in range(NC):
                    s2p = ps.tile([128, 512], F32, tag="mm")
                    for g in range(NG):
                        gp, gi = g // 2, g % 2
                        nc.tensor.matmul(
                            s2p[0:1, :], ev8[:, g:g + 1, 0:1], E[:, gp, ch, gi, :],
                            start=(g == 0), stop=(g == NG - 1))
                    nc.any.tensor_copy(s2row[0:1, ch * 512:(ch + 1) * 512],
                                       s2p[0:1, :])
                pts2 = pt.tile([128, 512], F32, tag="tr")
                for t in range(NT):
                    nc.tensor.transpose(pts2[:, t:t + 1],
                                        s2row[0:1, t * 128:(t + 1) * 128],
                                        ident_f[0:1, 0:1])

                # ---- qN8[j%128, kd, g, :] = q * invs1 * SQ (fp8) ----
                qN8 = sb.tile([128, NK, NG, 128], FP8, tag="qN8")
                for g in range(NG):
                    nc.vector.tensor_scalar_mul(qN8[:, :, g, :], qB[:, g, :],
                                                invs1q[:, g:g + 1])

                # ---- AT = (q/s1).T @ E (DR); cAT = cTb * AT ----
                AT = sb.tile([128, NK, Lc], BF16, tag="AT")
                cAT = sb.tile([128, NK, Lc], BF16, tag="cAT")
                for kd in range(NK):
                    for ch in range(NC):
                        pm = ps.tile([128, 512], F32, tag="mm")
                        for gp in range(2):
                            nc.tensor.matmul(
                                pm, qN8[:, kd, 2 * gp:2 * gp + 2, :],
                                E[:, gp, ch, :, :],
                                start=(gp == 0), stop=(gp == 1), perf_mode=DRM)
                        sl = slice(ch * 512, (ch + 1) * 512)
                        nc.any.tensor_scalar_mul(AT[:, kd, sl], pm, 1.0 / SQ)
                        nc.any.tensor_mul(cAT[:, kd, sl], cTb[:, kd, sl], AT[:, kd, sl])

                # ---- G transposes + scale: G[i%128, jg, t, :] = E^T/s2*SG ----
                invs2c = sb.tile([128, NT], F32, tag="invs2c")
                nc.vector.reciprocal(out=invs2c, in_=pts2[:, 0:NT])
                nc.vector.tensor_scalar_mul(invs2c, invs2c, SG * SEV)
                G = sb.tile([128, NG, NT, 128], FP8, tag="G")
                for tp in range(NT // 2):
                    p8t = p8.tile([128, NG, 2, 128, 2], FP8, tag="tr8")
                    for tl in range(2):
                        t = 2 * tp + tl
                        ch, off = t // 4, (t % 4) * 128
                        for g in range(NG):
                            gp, gi = g // 2, g % 2
                            nc.tensor.transpose(
                                p8t[:, g, tl, :, 0],
                                E[:, gp, ch, gi, off:off + 128], ident8)
                    for tl in range(2):
                        t = 2 * tp + tl
                        nc.any.tensor_scalar_mul(G[:, :, t, :],
                                                 p8t[:, :, tl, :, 0],
                                                 invs2c[:, t:t + 1])

                # ---- Y8 = (G.T @ c8) * ev/s1 * SY (DR) ----
                ysc = sb.tile([128, NG], F32, tag="ysc")
                nc.vector.tensor_mul(ysc, ev_colf, invs1)
                nc.vector.tensor_scalar_mul(ysc, ysc, SY / SG)
                Y8 = sb.tile([128, NK, NG, 128], FP8, tag="Y8")
                for g in range(NG):
                    pm = ps.tile([128, 512], F32, tag="mm")
                    for tp in range(NT // 2):
                        nc.tensor.matmul(
                            pm, G[:, g, 2 * tp:2 * tp + 2, :],
                            c8[:, 2 * tp:2 * tp + 2, :],
                            start=(tp == 0), stop=(tp == NT // 2 - 1),
                            perf_mode=DRM)
                    nc.vector.tensor_scalar_mul(Y8[:, :, g, :], pm,
                                                ysc[:, g:g + 1])

                # ---- BmT = Y.T @ E (DR); cBmT = cTb * BmT ----
                BmT = sb.tile([128, NK, Lc], BF16, tag="BmT")
                for kd in range(NK):
                    for ch in range(NC):
                        pm = ps.tile([128, 512], F32, tag="mm")
                        for gp in range(2):
                            nc.tensor.matmul(
                                pm, Y8[:, kd, 2 * gp:2 * gp + 2, :],
                                E[:, gp, ch, :, :],
                                start=(gp == 0), stop=(gp == 1), perf_mode=DRM)
                        sl = slice(ch * 512, (ch + 1) * 512)
                        nc.any.tensor_scalar_mul(BmT[:, kd, sl], pm, 1.0 / SY)
                        nc.any.tensor_mul(BmT[:, kd, sl], BmT[:, kd, sl], cTb[:, kd, sl])

                # ---- out = c@W1 + A@W2 + cA@W3 + cB@W4 + br ----
                for t2 in range(NT // 2):
                    pm = ps.tile([128, 1024], F32, tag="mm")
                    for half in range(2):
                        t = t2 * 2 + half
                        first = True
                        for si, src in enumerate((cTb, AT, cAT, BmT)):
                            for kd in range(NK):
                                nc.tensor.matmul(
                                    pm[:, half * 512:(half + 1) * 512],
                                    src[:, kd, t * 128:(t + 1) * 128],
                                    W_sb[:, si * NK + kd, :],
                                    start=first, stop=(si == 3 and kd == NK - 1))
                                first = False
                    ot = sb.tile([128, 2, 512], F32, tag="outst", bufs=3)
                    for half in range(2):
                        nc.any.tensor_add(ot[:, half, :],
                                          pm[:, half * 512:(half + 1) * 512], br_bc)
                    nc.sync.dma_start(
                        out=out2[b].rearrange("(u p) e -> p u e", p=128)[:, t2 * 2:t2 * 2 + 2, :],
                        in_=ot)

            if repeat > 1:
                hints = (mybir.EngineType.PE, mybir.EngineType.DVE,
                         mybir.EngineType.Activation, mybir.EngineType.SP,
                         mybir.EngineType.Pool)
                with tc.For_i(0, repeat, 1, hint_engines=hints):
                    for b in range(BPC):
                        one_batch(b, tiles0 if b == 0 else issue_loads(b))
            else:
                for b in range(BPC):
                    one_batch(b, tiles0 if b == 0 else issue_loads(b))

    nc.compile()
    return nc


class Runner:
    """Persistent SPMD runner: jit once, execute many times."""

    def __init__(self, nc):
        import jax
        from jax.experimental.shard_map import shard_map
        from jax.sharding import Mesh, PartitionSpec

        bass2jax.install_neuronx_cc_hook()
        self.nc = nc
        self.jax = jax

        partition_name = (
            nc.partition_id_tensor.name if nc.partition_id_tensor else None
        )
        in_names, out_names, out_avals, zero_shapes = [], [], [], []
        for alloc in nc.m.functions[0].allocations:
            if not isinstance(alloc, mybir.MemoryLocationSet):
                continue
            name = alloc.memorylocations[0].name
            if alloc.kind == "ExternalInput":
                if name != partition_name:
                    in_names.append(name)
            elif alloc.kind == "ExternalOutput":
                shape = tuple(alloc.tensor_shape)
                dtype = mybir.dt.np(alloc.dtype)
                out_names.append(name)
                out_avals.append(jax.core.ShapedArray(shape, dtype))
                zero_shapes.append((shape, dtype))
        self.in_names = list(in_names)
        self.out_names = out_names
        self.out_avals = out_avals
        self.zero_shapes = zero_shapes
        n_params = len(in_names)
        n_outs = len(out_names)

        all_in_names = list(in_names) + list(out_names)
        if partition_name is not None:
            all_in_names.append(partition_name)

        def _body(*args):
            operands = list(args)
            if partition_name is not None:
                operands.append(bass2jax.partition_id_tensor())
            outs = bass2jax._bass_exec_p.bind(
                *operands,
                out_avals=tuple(out_avals),
                in_names=tuple(all_in_names),
                out_names=tuple(out_names),
                lowering_input_output_aliases=(),
                sim_require_finite=True,
                sim_require_nnan=True,
                nc=nc,
            )
            return tuple(outs)

        devices = jax.devices()[:N_CORES]
        mesh = Mesh(np.asarray(devices), ("core",))
        in_specs = (PartitionSpec("core"),) * (n_params + n_outs)
        out_specs = (PartitionSpec("core"),) * n_outs
        self.fn = jax.jit(
            shard_map(_body, mesh=mesh, in_specs=in_specs,
                      out_specs=out_specs, check_rep=False),
            keep_unused=True,
        )

    def concat_inputs(self, in_maps):
        return [
            np.concatenate([np.asarray(m[name]) for m in in_maps], axis=0)
            for name in self.in_names
        ]

    def zeros(self):
        return [
            np.zeros((N_CORES * s[0], *s[1:]), d) for (s, d) in self.zero_shapes
        ]

    def run_device(self, concat_in, zeros):
        out = self.fn(*concat_in, *zeros)
        self.jax.block_until_ready(out)
        return out

    def run(self, in_maps):
        outs = self.run_device(self.concat_inputs(in_maps), self.zeros())
        return [
            {
                name: np.asarray(outs[i]).reshape(
                    N_CORES, *self.out_avals[i].shape)[c]
                for i, name in enumerate(self.out_names)
            }
            for c in range(N_CORES)
        ]


_CACHED = {}


def _get_runner(**kw):
    key = tuple(sorted(kw.items()))
    if key not in _CACHED:
        _CACHED[key] = Runner(build_program(**kw))
    return _CACHED[key]


def make_in_maps(context, question, w0, wr, br):
    return [
        {
            "c2": context[c * BPC:(c + 1) * BPC],
            "q2": question[c * BPC:(c + 1) * BPC],
            "w0": w0,
            "wr": wr,
            "br": br,
        }
        for c in range(N_CORES)
    ]


def kernel(context, question, w0, wr, br):
    context = np.ascontiguousarray(np.asarray(context, dtype=np.float32))
    question = np.ascontiguousarray(np.asarray(question, dtype=np.float32))
    w0 = np.ascontiguousarray(np.asarray(w0, dtype=np.float32))
    wr = np.ascontiguousarray(np.asarray(wr, dtype=np.float32))
    br = np.ascontiguousarray(np.asarray(br, dtype=np.float32))

    runner = _get_runner()
    res = runner.run(make_in_maps(context, question, w0, wr, br))
    return np.concatenate([res[c]["out2"] for c in range(N_CORES)], axis=0)
